# revision 3
# baseline (speedup 1.0000x reference)
"""CKSAAP embedding kernel for Trainium2 (8 NeuronCores, data-parallel over batch).

Strategy per (sequence, gap t):
    hist[d, bin] = sum_i vals_t[i, d] * onehot(idx_t[i])[bin]
computed as 16 accumulating PE matmuls (K=128 positions per chunk,
stationary = vals chunk [128, 64] fp16, moving = one-hot [128, 400] fp16,
accumulated fp32 in PSUM). One-hots are built on-chip from an iota row
compared against the per-position pair index (exact 0/1 in fp16):
  - VectorE: tensor_scalar(is_equal) with per-partition scalar
  - ScalarE: Abs(iota - idx) then Relu(1 - .) (exact for integer values)
vals_t = emb + shift(emb, t+1) built from partition-shifted SBUF copies
(DMA) + one VectorE add; the 0.5/(L-t-1) scale is folded into the final
PSUM->SBUF evacuation on ScalarE.

Host side: shards batch 256 -> 8 cores x 32 seqs, precasts emb to fp16 in
chunk-major layout, precomputes pair indices (seq*20 + shifted seq, -1 for
out-of-range tail), and transposes the device output [b,t,64,400] to the
reference layout [b,t,20,20,64].
"""

import numpy as np

from concourse import bacc, mybir
from concourse.bass_utils import run_bass_kernel_spmd
from concourse.tile import TileContext

NCORES = 8
B, L, D = 256, 2048, 64
NSEQ = B // NCORES  # 32 sequences per core
P = 128
NCH = L // P  # 16 position chunks per sequence
KP1 = 4  # gaps t = 0..3
NBINS = 400
F16 = mybir.dt.float16
F32 = mybir.dt.float32

# fraction pattern for one-hot engine choice: every act_mod-th one-hot goes
# to ScalarE (2 activation ops), the rest to VectorE (1 tensor_scalar op).
ACT_MOD = 3


def build_program(nseq=NSEQ, act_mod=ACT_MOD):
    nc = bacc.Bacc()
    emb16 = nc.declare_dram_parameter("emb16", [nseq, P, NCH * D], F16, False)
    idxp = nc.declare_dram_parameter("idxp", [nseq, P, KP1 * NCH], F32, False)
    idxn = nc.declare_dram_parameter("idxn", [nseq, P, KP1 * NCH], F32, False)
    iota = nc.declare_dram_parameter("iota", [P, NBINS], F16, False)
    hist = nc.declare_dram_parameter("hist", [nseq, KP1, D, NBINS], F32, True)

    with TileContext(nc) as tc:
        with (
            tc.tile_pool(name="const", bufs=1) as constp,
            tc.tile_pool(name="emb", bufs=2) as embp,
            tc.tile_pool(name="oh", bufs=12) as ohp,
            tc.tile_pool(name="ps", bufs=2, space="PSUM") as psp,
            tc.tile_pool(name="outs", bufs=8) as outsp,
        ):
            iota_t = constp.tile([P, NBINS], F16)
            nc.sync.dma_start(out=iota_t[:], in_=iota[:])

            ohctr = 0
            for b in range(nseq):
                embA = embp.tile([P, NCH * D], F16, tag="embA")
                nc.sync.dma_start(out=embA[:], in_=emb16[b])
                idxP = embp.tile([P, KP1 * NCH], F32, tag="idxP")
                nc.sync.dma_start(out=idxP[:], in_=idxp[b])
                idxN = embp.tile([P, KP1 * NCH], F32, tag="idxN")
                nc.sync.dma_start(out=idxN[:], in_=idxn[b])

                vals = []
                for t in range(KP1):
                    s = t + 1
                    sh = embp.tile([P, NCH * D], F16, tag=f"sh{t}")
                    # positions >= L get finite filler (masked by idx == -1).
                    # Engine ops need 32-aligned base partitions, so zero the
                    # whole 32-row tail block first; the shift DMAs below
                    # overwrite the in-range rows.
                    nc.vector.memset(sh[P - 32 : P, (NCH - 1) * D : NCH * D], 0.0)
                    # positions 128c+p+s: partitions p<128-s from chunk c,
                    # partitions p>=128-s wrap into chunk c+1.
                    nc.sync.dma_start(out=sh[0 : P - s, :], in_=embA[s:P, :])
                    nc.sync.dma_start(
                        out=sh[P - s : P, 0 : (NCH - 1) * D],
                        in_=embA[0:s, D : NCH * D],
                    )
                    v = embp.tile([P, NCH * D], F16, tag=f"v{t}")
                    nc.vector.tensor_tensor(
                        out=v[:], in0=embA[:], in1=sh[:], op=mybir.AluOpType.add
                    )
                    vals.append(v)

                pss = [
                    psp.tile(
                        [D, NBINS], F32, tag=f"ps{t}", space="PSUM", name=f"ps{t}_{b}"
                    )
                    for t in range(KP1)
                ]
                for c in range(NCH):
                    for t in range(KP1):
                        col = t * NCH + c
                        oh = ohp.tile([P, NBINS], F16, tag="oh")
                        ohctr += 1
                        if act_mod and ohctr % act_mod == 0:
                            tmp = ohp.tile([P, NBINS], F16, tag="ohtmp")
                            nc.scalar.activation(
                                out=tmp[:],
                                in_=iota_t[:],
                                func=mybir.ActivationFunctionType.Abs,
                                bias=idxN[:, col : col + 1],
                                scale=1.0,
                            )
                            nc.scalar.activation(
                                out=oh[:],
                                in_=tmp[:],
                                func=mybir.ActivationFunctionType.Relu,
                                bias=1.0,
                                scale=-1.0,
                            )
                        else:
                            nc.vector.tensor_scalar(
                                out=oh[:],
                                in0=iota_t[:],
                                scalar1=idxP[:, col : col + 1],
                                scalar2=None,
                                op0=mybir.AluOpType.is_equal,
                            )
                        nc.tensor.matmul(
                            out=pss[t][:],
                            lhsT=vals[t][:, c * D : (c + 1) * D],
                            rhs=oh[:],
                            start=(c == 0),
                            stop=(c == NCH - 1),
                        )
                for t in range(KP1):
                    st = outsp.tile([D, NBINS], F32, tag="st")
                    nc.scalar.mul(out=st[:], in_=pss[t][:], mul=float(0.5 / (L - t - 1)))
                    nc.sync.dma_start(out=hist[b, t], in_=st[:])

    nc.compile()
    return nc


def host_prep(seq, emb, nseq_total=B):
    """Full-batch host-side input prep (cheap integer/cast work only)."""
    s = np.asarray(seq).astype(np.int64)
    e = np.asarray(emb, dtype=np.float32).astype(np.float16)
    n_b = s.shape[0]
    emb16 = np.ascontiguousarray(
        e.reshape(n_b, NCH, P, D).transpose(0, 2, 1, 3)
    ).reshape(n_b, P, NCH * D)
    idx = np.full((n_b, KP1, L), -1.0, np.float32)
    for t in range(KP1):
        n = L - t - 1
        idx[:, t, :n] = (s[:, :n] * 20 + s[:, t + 1 : t + 1 + n]).astype(np.float32)
    idxp = np.ascontiguousarray(
        idx.reshape(n_b, KP1, NCH, P).transpose(0, 3, 1, 2)
    ).reshape(n_b, P, KP1 * NCH)
    idxn = np.ascontiguousarray(-idxp)
    iota = np.ascontiguousarray(
        np.broadcast_to(np.arange(NBINS, dtype=np.float16), (P, NBINS))
    )
    return emb16, idxp, idxn, iota


_prog_cache = {}


def get_program(nseq=NSEQ, act_mod=ACT_MOD):
    key = (nseq, act_mod)
    if key not in _prog_cache:
        _prog_cache[key] = build_program(nseq, act_mod)
    return _prog_cache[key]


def make_in_maps(emb16, idxp, idxn, iota, nseq=NSEQ, ncores=NCORES):
    in_maps = []
    for ci in range(ncores):
        sl = slice(ci * nseq, (ci + 1) * nseq)
        in_maps.append(
            {
                "emb16": np.ascontiguousarray(emb16[sl]),
                "idxp": np.ascontiguousarray(idxp[sl]),
                "idxn": np.ascontiguousarray(idxn[sl]),
                "iota": iota,
            }
        )
    return in_maps


def postprocess(hists):
    # hists: [n_b, KP1, D, NBINS] -> [n_b, KP1, 20, 20, D]
    n_b = hists.shape[0]
    return np.ascontiguousarray(
        hists.transpose(0, 1, 3, 2).reshape(n_b, KP1, 20, 20, D)
    ).astype(np.float32)


def kernel(seq, emb, k):
    assert int(k) == 3, "kernel hardcodes k=3"
    seq = np.asarray(seq)
    emb = np.asarray(emb)
    assert seq.shape == (B, L) and emb.shape == (B, L, D)
    emb16, idxp, idxn, iota = host_prep(seq, emb)
    nc = get_program()
    in_maps = make_in_maps(emb16, idxp, idxn, iota)
    res = run_bass_kernel_spmd(nc, in_maps, list(range(NCORES)))
    hists = np.concatenate(
        [res.results[ci]["hist"] for ci in range(NCORES)], axis=0
    )
    return postprocess(hists)
